# revision 23
# baseline (speedup 1.0000x reference)
"""BLIP3o DiT block on 8 Trainium2 NeuronCores.

Strategy: data-parallel over batch (32 batches -> 4 per core), zero collectives.
On-chip layout: activations live transposed [feature, token]; all matmul
operands are bf16 (1 cycle/row on the PE, half-size LDWEIGHTS, half HBM
traffic), accumulation stays fp32 in PSUM.  Residual stream (hsT),
modulation, rms statistics and rope tables stay fp32.

Overlap structure: each rms-norm's square+mean-square matmuls are issued
inside the preceding phase's consumers (mod loop for rms1, o1 for rms2, o2
for rms3) so only the short sqrt->recip->normalize tail sits between phases.
The final gate/residual/store is folded into the down1 PSUM drain.  Weight
stationaries are reused across both token halves (f-outer loops).

Per-core dataflow (T = 4*256 = 1024 tokens):
  modT  = (silu(temb) @ ada_w + ada_b)^T          (tiny-N matmuls, o-chunked)
  x1T   = rope(rms(hsT)*(1+sc_msa)+sh_msa)        (bf16)
  V1    = x1T-stationary @ wv1 (natural layout)
  QT,KT = wq1/wk1-stationary @ x1T
  attn1 : scoresT = KT-slice-stationary @ QT (both kc chunks into one PSUM
          bank); single exp per head on ScalarE -> bf16; AV + ones-denominator
  h1    = hsT + g_msa * (wo1-proj of attnout)      (in place)
  evaT  = eva_w-stationary @ encT + eva_b          (encT streamed in halves)
  attn2 : k2/v2 from evaT ; q2 from rms(h1,n2) ; no gate on residual
  yT    = rms(h2,n3)*(1+sc_mlp)+sh_mlp
  mlp   = (silu(gateT) * upT) @ down               (I split in halves)
  out   = h2 + g_mlp * mlp                         (folded into down1 drain)
"""
import os
import sys
import numpy as np

if "/root/pylocal" not in sys.path:
    sys.path.insert(0, "/root/pylocal")  # antenv.axon_hooks shim (NTFF tracing)
try:
    import antenv
    if "/root/pylocal/antenv" not in list(antenv.__path__):
        antenv.__path__.append("/root/pylocal/antenv")
except Exception:
    pass

import concourse.bass as bass
from concourse import bacc
import concourse.mybir as mybir
from concourse.tile import TileContext
from concourse.bass_utils import run_bass_kernel_spmd

F32 = mybir.dt.float32
BF16 = mybir.dt.bfloat16
AF = mybir.ActivationFunctionType
OP = mybir.AluOpType
BF16NP = mybir.dt.np(mybir.dt.bfloat16)

B, S, L, H, NH, HD, I, E = 32, 256, 256, 1024, 16, 64, 4096, 4096
EPS = 1e-6
GRID = 16
NC_ = 8            # cores
BPC = B // NC_     # batches per core = 4
T = BPC * S        # tokens per core = 1024
HC = H // 128      # 8 feature chunks
EC = E // 128      # 32
IC = I // 128      # 32
NCONST = 3 * HC + HC + 48 + 1 + 192   # n1T n2T n3T eva_bT ada_bT eps ada_bT_x4


def _rope_tables():
    q = H // 4
    inv = 1.0 / (10000.0 ** (np.arange(0, q, dtype=np.float64)[::2] / q))  # [128]
    qd = 128
    pos_x = np.repeat(np.arange(GRID, dtype=np.float64), GRID)  # [S]
    pos_y = np.tile(np.arange(GRID, dtype=np.float64), GRID)
    fx = pos_x[:, None] * inv[None, :qd]   # [S, 128]
    fy = pos_y[:, None] * inv[None, :qd]
    t = lambda a: np.ascontiguousarray(
        np.tile(a.T.astype(np.float32), (1, BPC)))  # [128, S] -> [128, T]
    return t(np.cos(fx)), t(np.sin(fx)), t(np.cos(fy)), t(np.sin(fy))


def build_program():
    nc = bacc.Bacc()

    # ---------------- DRAM params ----------------
    d = {}
    def P(name, shape, dt, out=False):
        d[name] = nc.declare_dram_parameter(name, list(shape), dt, isOutput=out)
        return d[name]

    hsT_d = P("hsT", [H, T], F32)
    encT_d = P("encT", [E, T], BF16)
    P("tembT", [H, BPC], F32)
    for w in ["wq1", "wk1", "wv1", "wo1", "wq2", "wk2", "wv2", "wo2"]:
        P(w, [H, H], BF16)
    P("eva_w", [E, H], BF16)
    P("ada_w", [H, 6 * H], BF16)
    P("gate_w", [H, I], BF16)
    P("up_w", [H, I], BF16)
    P("down_w", [I, H], BF16)
    P("constsF", [128, NCONST], F32)   # n1T | n2T | n3T | eva_bT | ada_bT | eps
    for tb in ["cxt", "sxt", "cyt", "syt"]:
        P(tb, [128, T], F32)
    P("ones", [128, 128], BF16)
    outT_d = P("outT", [H, T], F32, out=True)

    r3 = lambda ap: ap.rearrange("(c p) t -> p c t", p=128)

    tc_cm = TileContext(nc)
    tc = tc_cm.__enter__()

    open_pools = {}

    def pool(name, bufs=1, side="left"):
        p = tc.alloc_tile_pool(name=name, bufs=bufs, side=side)
        open_pools[name] = p
        return p

    def free(name):
        open_pools.pop(name).release()

    # long-lived small pools
    wpool = pool("wstream", bufs=4)         # tag "w8": [128, 8, 128] slots
    sml = pool("sml", bufs=1)               # resid/gsil tags
    const = pool("const", bufs=1)
    ps_proj = tc.alloc_tile_pool(name="ps_proj", bufs=2, space="PSUM")
    ps_sc = tc.alloc_tile_pool(name="ps_sc", bufs=2, space="PSUM")
    ps_av = tc.alloc_tile_pool(name="ps_av", bufs=2, space="PSUM")

    # ---------------- constants (one DMA) + temb first ----------------
    tembT_sb = const.tile([128, HC, BPC], F32)
    nc.sync.dma_start(tembT_sb[:], d["tembT"].rearrange("(c p) b -> p c b", p=128))
    cst = const.tile([128, NCONST], F32)
    nc.sync.dma_start(cst[:], d["constsF"][:])
    n_sb = {"n1T": cst[:, 0:8], "n2T": cst[:, 8:16], "n3T": cst[:, 16:24],
            "eva_bT": cst[:, 24:32], "ada_bT": cst[:, 32:80]}
    eps_sb = cst[:, 80:81]
    ada_bx4 = cst[:, 81:81 + 192].rearrange("p (o b) -> p o b", b=BPC)
    ones_sb = const.tile([128, 128], BF16)
    nc.sync.dma_start(ones_sb[:], d["ones"][:])

    modT = const.tile([128, 48, BPC], F32)      # 6 splits x 8 chunks
    scale1 = const.tile([128, HC, BPC], F32)    # n1*(1+sc_msa)
    scale3 = const.tile([128, HC, BPC], F32)    # n3*(1+sc_mlp)

    p_hs = pool("p_hs")
    hsT = p_hs.tile([128, HC, T], F32)          # becomes h1T, then h2T in place
    hs_r = r3(hsT_d)
    p_rope = pool("p_rope")
    rope_t = {}
    for tb in ["cxt", "sxt", "cyt", "syt"]:
        rope_t[tb] = p_rope.tile([128, T], F32, name=tb + "_sb")

    # ---------------- rms machinery ----------------
    def rms_begin(name, x_sb, side="left"):
        """Returns (issue, finish).  issue(c) squares chunk c and accumulates
        the mean-square matmuls; call it from inside the preceding phase.
        finish(consumer) runs sqrt->recip->normalize and feeds consumer."""
        rtmp = pool("rtmp_" + name, side=side)
        ps_ms = tc.alloc_tile_pool(name="ps_ms_" + name, bufs=1, space="PSUM")
        ms = [ps_ms.tile([128, 512], F32, name=f"ms_{name}_{t}")
              for t in range(2)]

        def issue(c, on_vector=False):
            sq = rtmp.tile([128, T], BF16, tag="sq", bufs=2, name=f"sq_{name}")
            if on_vector:
                nc.vector.tensor_tensor(sq[:], x_sb[:, c], x_sb[:, c], OP.mult)
            else:
                nc.scalar.activation(sq[:], x_sb[:, c], AF.Square)
            for t in range(2):
                nc.tensor.matmul(ms[t][:], ones_sb[:],
                                 sq[:, t * 512:(t + 1) * 512],
                                 start=(c == 0), stop=(c == HC - 1))

        def finish_half(t, consumer):
            sroot = rtmp.tile([128, 512], F32, tag="sroot", bufs=2,
                              name=f"sroot_{name}")
            nc.scalar.activation(sroot[:], ms[t][:], AF.Sqrt,
                                 bias=eps_sb, scale=1.0 / H)
            invn = rtmp.tile([128, 512], F32, tag="invn", bufs=2,
                             name=f"invn_{name}")
            nc.vector.reciprocal_approx_fast(invn[:], sroot[:])
            for c in range(HC):
                xn = rtmp.tile([128, 512], F32, tag="xn", bufs=4,
                               name=f"xn_{name}")
                nc.vector.tensor_tensor(xn[:],
                                        x_sb[:, c, t * 512:(t + 1) * 512],
                                        invn[:], OP.mult)
                consumer(c, t, xn)

        def release():
            ps_ms.release()
            free("rtmp_" + name)

        def finish(consumer):
            finish_half(0, consumer)
            finish_half(1, consumer)
            release()

        return issue, finish, finish_half, release

    # x1T / rope staging (rms1 consumers write here during the mod loop)
    p_x1 = pool("p_x1", side="right")
    x1T = p_x1.tile([128, HC, T], BF16)
    p_xm = pool("p_xm")
    xm = [p_xm.tile([128, T], F32, name=f"xm{i}") for i in range(4)]

    def rms1_consumer(c, t, xn):
        dst = xm[c] if c < 4 else x1T[:, c]
        for b2 in range(2):
            b = 2 * t + b2
            nc.vector.tensor_scalar(dst[:, b * S:(b + 1) * S],
                                    xn[:, b2 * S:(b2 + 1) * S],
                                    scale1[:, c, b:b + 1],
                                    modT[:, 0 + c, b:b + 1],
                                    OP.mult, OP.add)

    r1_issue, r1_finish, r1_half, r1_release = rms_begin("r1", hsT, side="right")

    # ---------------- phase 0: mod loop with interleaved input DMAs ----------
    # mod bias-adds are done in two bulk vector ops (ada_bx4); the per-pair
    # PSUM drain is a scalar copy so the vector queue never stalls the PE.
    rp_holder = {}

    def rope_pair(pi):
        rp = rp_holder["rp"]
        (i0, i1, ct, st) = [(0, 1, "cxt", "sxt"), (2, 3, "cyt", "syt")][pi]
        a, bb = xm[i0], xm[i1]
        t1 = rp.tile([128, T], F32, tag="t1", bufs=2, name="t1")
        t2 = rp.tile([128, T], F32, tag="t2", bufs=2, name="t2")
        nc.vector.tensor_tensor(t1[:], a[:], rope_t[ct][:], OP.mult)
        nc.gpsimd.tensor_tensor(t2[:], bb[:], rope_t[st][:], OP.mult)
        nc.vector.tensor_tensor(x1T[:, i0], t1[:], t2[:], OP.subtract)
        t3 = rp.tile([128, T], F32, tag="t1", bufs=2, name="t3")
        t4 = rp.tile([128, T], F32, tag="t2", bufs=2, name="t4")
        nc.gpsimd.tensor_tensor(t3[:], a[:], rope_t[st][:], OP.mult)
        nc.vector.tensor_tensor(t4[:], bb[:], rope_t[ct][:], OP.mult)
        nc.vector.tensor_tensor(x1T[:, i1], t3[:], t4[:], OP.add)

    with nc.named_scope("mod"):
        stemb = const.tile([128, HC, BPC], BF16)
        nc.scalar.activation(stemb[:], tembT_sb[:], AF.Silu)
        ada_r = r3(d["ada_w"])  # [128, 8, 6144]
        mp = None
        for o in range(48):
            if o < HC:
                nc.sync.dma_start(hsT[:, o], hs_r[:, o])
            elif o < 12:
                tb = ["cxt", "sxt", "cyt", "syt"][o - 8]
                nc.sync.dma_start(rope_t[tb][:], d[tb][:])
            wt = wpool.tile([128, HC, 128], BF16, tag="w8", name="ada_t")
            nc.sync.dma_start(wt[:], ada_r[:, :, o * 128:(o + 1) * 128])
            if o % 2 == 0:
                mp = ps_sc.tile([128, 2, BPC], F32, tag="sc", name="mod_ps")
            for f in range(HC):
                nc.tensor.matmul(mp[:, o % 2], wt[:, f], stemb[:, f],
                                 start=(f == 0), stop=(f == HC - 1))
            if o % 2 == 1:
                nc.scalar.copy(modT[:, o - 1:o + 1, :], mp[:])
            if 8 <= o < 16:
                r1_issue(o - 8, on_vector=True)
            if o == 15:
                nc.vector.tensor_tensor(modT[:, 0:16], modT[:, 0:16],
                                        ada_bx4[:, 0:16], OP.add)
                for c in range(HC):
                    nc.vector.tensor_scalar(scale1[:, c], modT[:, 8 + c], 1.0,
                                            n_sb["n1T"][:, c:c + 1],
                                            OP.add, OP.mult)
            elif o == 17:
                with nc.named_scope("rms1"):
                    r1_half(0, rms1_consumer)
            elif o == 25:
                with nc.named_scope("rms1"):
                    r1_half(1, rms1_consumer)
                r1_release()
            elif o == 33:
                with nc.named_scope("rope"):
                    rp_holder["rp"] = pool("p_ropetmp")
                    rope_pair(0)
            elif o == 41:
                with nc.named_scope("rope"):
                    rope_pair(1)
                    free("p_ropetmp")
                free("p_xm")
                free("p_rope")
        nc.vector.tensor_tensor(modT[:, 16:48], modT[:, 16:48],
                                ada_bx4[:, 16:48], OP.add)
        for c in range(HC):
            nc.vector.tensor_scalar(scale3[:, c], modT[:, 32 + c], 1.0,
                                    n_sb["n3T"][:, c:c + 1],
                                    OP.add, OP.mult)

    # ---------------- helpers ----------------
    def copy_px(idx, dst, src):
        nc.scalar.copy(dst, src)

    def proj_T(name, w_name, src_sb, KC, consumer, OC=HC, wtag="w8"):
        """Y^T accumulation: lhsT = weight chunks [128,KC,128], rhs = src
        [128,KC,T].  Stationary reused across both token halves."""
        w_r = r3(d[w_name])
        with nc.named_scope(name):
            for o in range(OC):
                wt = wpool.tile([128, KC, 128], BF16, tag=wtag, name=f"{name}_w")
                nc.sync.dma_start(wt[:], w_r[:, :, o * 128:(o + 1) * 128])
                for t in range(2):
                    p = ps_proj.tile([128, 512], F32, tag="proj", name=f"{name}_ps")
                    for f in range(KC):
                        nc.tensor.matmul(p[:], wt[:, f],
                                         src_sb[:, f, t * 512:(t + 1) * 512],
                                         start=(f == 0), stop=(f == KC - 1))
                    consumer(o, t, p)

    def copy_act(dst):
        def c(o, t, p):
            copy_px(o + t, dst[:, o, t * 512:(t + 1) * 512], p[:])
        return c

    def vnat(w_name, src_sb, dst_v, scope, side="left"):
        """V natural [toks(128-chunks), H] from src-stationary matmuls."""
        w_r = r3(d[w_name])  # [128, HC, H]
        wv = pool("wv_" + scope, bufs=(1 if scope == "v2" else 2), side=side)
        with nc.named_scope(scope):
            for oh in range(2):
                wt = wv.tile([128, HC, 512], BF16, tag="wvnat", name=f"{scope}_w")
                nc.sync.dma_start(wt[:], w_r[:, :, oh * 512:(oh + 1) * 512])
                for t in range(2 * BPC):
                    p = ps_proj.tile([128, 512], F32, tag="proj", name=f"{scope}_ps")
                    KC = src_sb.shape[1]
                    for f in range(KC):
                        nc.tensor.matmul(p[:], src_sb[:, f, t * 128:(t + 1) * 128],
                                         wt[:, f], start=(f == 0), stop=(f == KC - 1))
                    copy_px(oh + t, dst_v[:, t, oh * 512:(oh + 1) * 512], p[:])
        free("wv_" + scope)

    def attention(qt_sb, kt_sb, vp_sb, out_sb, scope):
        attnp = pool("attnp_" + scope, bufs=3, side="right")
        ps_den = tc.alloc_tile_pool(name="ps_den_" + scope, bufs=2, space="PSUM")
        with nc.named_scope(scope):
            for b in range(BPC):
                for hc in range(NH // 2):
                    # head pair (2*hc, 2*hc+1) lives at row groups 0-63 / 64-127
                    # of chunk hc; their score MMs use disjoint PE quadrants.
                    qs = [qt_sb[ho * 64:(ho + 1) * 64, hc, b * S:(b + 1) * S]
                          for ho in range(2)]
                    at = []
                    for ho in range(2):
                        sc_ps = ps_sc.tile([128, 2, S], F32, tag="sc",
                                           name="sc_ps")
                        for kc in range(2):
                            nc.tensor.matmul(
                                sc_ps[:, kc],
                                kt_sb[ho * 64:(ho + 1) * 64, hc,
                                      b * S + kc * 128: b * S + (kc + 1) * 128],
                                qs[ho], start=True, stop=True)
                        a = attnp.tile([128, 2, S], BF16, tag=f"attn{ho}",
                                       name="attn_sb")
                        nc.scalar.activation(a[:], sc_ps[:], AF.Exp,
                                             scale=float(HD) ** -0.5)
                        at.append(a)
                    for ho in range(2):
                        h = 2 * hc + ho
                        av = ps_av.tile([64, S], F32, tag="av", name="av_ps")
                        for kc in range(2):
                            nc.tensor.matmul(av[:],
                                             vp_sb[:, b * 2 + kc,
                                                   h * 64:(h + 1) * 64],
                                             at[ho][:, kc],
                                             start=(kc == 0), stop=(kc == 1))
                        den = ps_den.tile([64, S], F32, tag="den", name="den_ps")
                        for kc in range(2):
                            nc.tensor.matmul(den[:], ones_sb[:, 0:64],
                                             at[ho][:, kc],
                                             start=(kc == 0), stop=(kc == 1))
                        inv = attnp.tile([64, S], F32, tag="inv", name="inv_sb")
                        nc.vector.reciprocal_approx_fast(inv[:], den[:])
                        nc.vector.tensor_tensor(
                            out_sb[ho * 64:(ho + 1) * 64, hc, b * S:(b + 1) * S],
                            av[:], inv[:], OP.mult)
        ps_den.release()
        free("attnp_" + scope)

    # ---------------- phase A: V, Q, K, attention, o1 ------------------------
    p_vp = pool("p_vp")
    vp = p_vp.tile([128, 2 * BPC, NH * 64], BF16)
    vnat("wv1", x1T, vp, "v1")

    p_qt = pool("p_qt"); qt = p_qt.tile([128, HC, T], BF16)
    p_kt = pool("p_kt"); kt = p_kt.tile([128, HC, T], BF16)
    proj_T("q1", "wq1", x1T, HC, copy_act(qt))
    proj_T("k1", "wk1", x1T, HC, copy_act(kt))
    free("p_x1")

    p_ao = pool("p_ao", side="right")
    attnout = p_ao.tile([128, HC, T], BF16)
    attention(qt, kt, vp, attnout, "attn1")
    free("p_kt"); free("p_qt"); free("p_vp")

    p_r2 = pool("p_r2")
    rms2T = p_r2.tile([128, HC, T], BF16)
    r2_issue, _, r2_half, r2_release = rms_begin("r2", hsT, side="right")

    def resid_gated(g_split, rms_issue=None):
        def c(o, t, p):
            tg = sml.tile([128, 512], F32, tag="resid", name="resid_t")
            for b2 in range(2):
                b = t * 2 + b2
                nc.vector.tensor_scalar(tg[:, b2 * S:(b2 + 1) * S],
                                        p[:, b2 * S:(b2 + 1) * S],
                                        modT[:, g_split * 8 + o, b:b + 1],
                                        None, OP.mult)
            nc.vector.tensor_tensor(hsT[:, o, t * 512:(t + 1) * 512],
                                    hsT[:, o, t * 512:(t + 1) * 512],
                                    tg[:], OP.add)
            if t == 1 and rms_issue is not None:
                rms_issue(o)
        return c

    proj_T("o1", "wo1", attnout, HC, resid_gated(2, r2_issue))

    # rms2 tail right after o1; its vector work overlaps eva's PE stream
    def rms2_consumer(c, t, xn):
        nc.vector.tensor_scalar(rms2T[:, c, t * 512:(t + 1) * 512], xn[:],
                                n_sb["n2T"][:, c:c + 1], None, OP.mult)

    with nc.named_scope("rms2"):
        r2_half(0, rms2_consumer)   # hsT now holds h1
        r2_half(1, rms2_consumer)

    # ---------------- phase B: evaT = eva_w-stat @ encT + b ------------------
    p_eva = pool("p_eva")
    evaT = p_eva.tile([128, HC, T], BF16)
    enc_r = r3(encT_d)  # [128, 32, T]
    w_r_eva = r3(d["eva_w"])  # [128, 32, 1024]
    with nc.named_scope("eva"):
        p_enc = pool("p_enc", bufs=3)
        wev = pool("p_weva", bufs=2)
        for th in range(2):
            # two encT quarter-tiles per half, each DMA'd once and reused by
            # all 8 o-chunks; bufs=3 lets the next half's first quarter prefetch.
            enq = []
            for tq2 in range(2):
                ench = p_enc.tile([128, EC, 256], BF16, tag="ench", name="ench")
                tq = th * 2 + tq2
                nc.sync.dma_start(ench[:], enc_r[:, :, tq * 256:(tq + 1) * 256])
                enq.append(ench)
            for o in range(HC):
                wt = wev.tile([128, EC, 128], BF16, tag="weva", name="eva_w_t")
                nc.sync.dma_start(wt[:], w_r_eva[:, :, o * 128:(o + 1) * 128])
                for tq2 in range(2):
                    tq = th * 2 + tq2
                    p = ps_proj.tile([128, 256], F32, tag="proj", name="eva_ps")
                    for f in range(EC):
                        nc.tensor.matmul(p[:], wt[:, f], enq[tq2][:, f],
                                         start=(f == 0), stop=(f == EC - 1))
                    nc.vector.tensor_scalar(evaT[:, o, tq * 256:(tq + 1) * 256],
                                            p[:], n_sb["eva_bT"][:, o:o + 1],
                                            None, OP.add)
        free("p_weva")
        free("p_enc")
    r2_release()
    free("p_ao")

    # ---------------- phase C: cross attention (k2, v2, q2) ------------------
    p_ao2 = pool("p_ao2", side="right")
    attn2out = p_ao2.tile([128, HC, T], BF16)
    p_k2 = pool("p_k2", side="right"); k2t = p_k2.tile([128, HC, T], BF16)
    proj_T("k2", "wk2", evaT, HC, copy_act(k2t))

    p_v2 = pool("p_v2", side="right")
    vp2 = p_v2.tile([128, 2 * BPC, NH * 64], BF16)
    vnat("wv2", evaT, vp2, "v2", side="right")
    free("p_eva")

    p_q2 = pool("p_q2", side="right"); q2t = p_q2.tile([128, HC, T], BF16)
    proj_T("q2", "wq2", rms2T, HC, copy_act(q2t))
    free("p_r2")

    attention(q2t, k2t, vp2, attn2out, "attn2")
    free("p_q2"); free("p_v2"); free("p_k2")

    # ---------------- phase D: rms3 + MLP ------------------------------------
    p_dacc = pool("p_dacc")
    dacc = p_dacc.tile([128, HC, T], F32)
    p_y = pool("p_y")
    yT = p_y.tile([128, HC, T], BF16)
    r3_issue, r3_finish, _, _ = rms_begin("r3", hsT, side="right")

    def resid_plain(o, t, p):
        nc.vector.tensor_tensor(hsT[:, o, t * 512:(t + 1) * 512],
                                hsT[:, o, t * 512:(t + 1) * 512], p[:], OP.add)
        if t == 1:
            r3_issue(o)

    proj_T("o2", "wo2", attn2out, HC, resid_plain)

    def rms3_consumer(c, t, xn):
        for b2 in range(2):
            b = 2 * t + b2
            nc.vector.tensor_scalar(yT[:, c, b * S:(b + 1) * S],
                                    xn[:, b2 * S:(b2 + 1) * S],
                                    scale3[:, c, b:b + 1],
                                    modT[:, 24 + c, b:b + 1],
                                    OP.mult, OP.add)

    with nc.named_scope("rms3"):
        r3_finish(rms3_consumer)   # hsT now holds h2
    free("p_ao2")

    gate_r = r3(d["gate_w"])  # [128, 8, 4096]
    up_r = r3(d["up_w"])
    down_r = r3(d["down_w"])  # [128, 32, 1024]
    out_r = r3(outT_d)
    IHC = IC // 2  # 16 I-chunks per half
    p_mlp = pool("p_mlp", side="right")
    wmlp = pool("p_wmlp", bufs=4)
    wdn = pool("p_wdown", bufs=2, side="right")
    for ih in range(2):
        mlpT = p_mlp.tile([128, IHC, T], BF16, tag="mlp", bufs=1, name="mlpT")
        with nc.named_scope(f"gateup{ih}"):
            for o in range(IHC):
                oc = ih * IHC + o
                wg = wmlp.tile([128, HC, 128], BF16, tag="w8", name="gate_w_t")
                nc.sync.dma_start(wg[:], gate_r[:, :, oc * 128:(oc + 1) * 128])
                wu = wmlp.tile([128, HC, 128], BF16, tag="w8", name="up_w_t")
                nc.sync.dma_start(wu[:], up_r[:, :, oc * 128:(oc + 1) * 128])
                for t in range(2):
                    pg = ps_proj.tile([128, 512], F32, tag="proj", name="g_ps")
                    for f in range(HC):
                        nc.tensor.matmul(pg[:], wg[:, f],
                                         yT[:, f, t * 512:(t + 1) * 512],
                                         start=(f == 0), stop=(f == HC - 1))
                    pu = ps_proj.tile([128, 512], F32, tag="proj", name="u_ps")
                    for f in range(HC):
                        nc.tensor.matmul(pu[:], wu[:, f],
                                         yT[:, f, t * 512:(t + 1) * 512],
                                         start=(f == 0), stop=(f == HC - 1))
                    gs = sml.tile([128, 512], BF16, tag="gsil", name="gsil")
                    nc.scalar.activation(gs[:], pg[:], AF.Silu)
                    nc.vector.tensor_tensor(mlpT[:, o, t * 512:(t + 1) * 512],
                                            gs[:], pu[:], OP.mult)
        with nc.named_scope(f"down{ih}"):
            for o in range(HC):
                wt = wdn.tile([128, IHC, 128], BF16, tag="wdown", name="down_w_t")
                nc.sync.dma_start(wt[:],
                                  down_r[:, ih * IHC:(ih + 1) * IHC,
                                         o * 128:(o + 1) * 128])
                for t in range(2):
                    p = ps_proj.tile([128, 512], F32, tag="proj", name="d_ps")
                    for f in range(IHC):
                        nc.tensor.matmul(p[:], wt[:, f],
                                         mlpT[:, f, t * 512:(t + 1) * 512],
                                         start=(f == 0), stop=(f == IHC - 1))
                    sl = slice(t * 512, (t + 1) * 512)
                    if ih == 0:
                        nc.vector.tensor_copy(dacc[:, o, sl], p[:])
                    else:
                        # fold the final out = h2 + g_mlp * mlp into the drain
                        nc.vector.tensor_tensor(dacc[:, o, sl],
                                                dacc[:, o, sl], p[:], OP.add)
                        for b2 in range(2):
                            b = 2 * t + b2
                            nc.vector.tensor_scalar(
                                dacc[:, o, b * S:(b + 1) * S],
                                dacc[:, o, b * S:(b + 1) * S],
                                modT[:, 40 + o, b:b + 1], None, OP.mult)
                        nc.vector.tensor_tensor(dacc[:, o, sl],
                                                dacc[:, o, sl],
                                                hsT[:, o, sl], OP.add)
                        nc.sync.dma_start(out_r[:, o, sl], dacc[:, o, sl])
    free("p_wdown")
    free("p_wmlp")
    free("p_mlp")
    free("p_y")

    for nm in reversed(list(open_pools)):
        free(nm)
    ps_av.release(); ps_sc.release(); ps_proj.release()
    tc_cm.__exit__(None, None, None)
    nc.compile()
    return nc


_CACHE = {}


def _get_program():
    if "nc" not in _CACHE:
        _CACHE["nc"] = build_program()
    return _CACHE["nc"]


def kernel(hidden_states, encoder_hidden_states, timestep_emb,
           wq1, wk1, wv1, wo1, wq2, wk2, wv2, wo2,
           eva_w, eva_b, ada_w, ada_b, gate_w, up_w, down_w, n1, n2, n3,
           _trace=False):
    nc = _get_program()
    f32 = lambda a: np.ascontiguousarray(np.asarray(a), dtype=np.float32)
    bf = lambda a: np.ascontiguousarray(np.asarray(a), dtype=np.float32).astype(BF16NP)

    cxt, sxt, cyt, syt = _rope_tables()
    colchunks = lambda v, n: np.asarray(v, np.float32).reshape(n, 128).T
    ada_bT = colchunks(ada_b, 48)
    constsF = np.concatenate([
        colchunks(n1, HC), colchunks(n2, HC), colchunks(n3, HC),
        colchunks(eva_b, HC), ada_bT,
        np.full((128, 1), EPS, np.float32),
        np.repeat(ada_bT, 4, axis=1)], axis=1)
    shared = dict(
        wq1=bf(wq1), wk1=bf(wk1), wv1=bf(wv1), wo1=bf(wo1),
        wq2=bf(wq2), wk2=bf(wk2), wv2=bf(wv2), wo2=bf(wo2),
        eva_w=bf(eva_w), ada_w=bf(ada_w), gate_w=bf(gate_w),
        up_w=bf(up_w), down_w=bf(down_w),
        constsF=np.ascontiguousarray(constsF),
        cxt=cxt, sxt=sxt, cyt=cyt, syt=syt,
        ones=np.ones((128, 128), BF16NP),
    )
    hs = f32(hidden_states)
    enc = f32(encoder_hidden_states)
    temb = f32(timestep_emb)

    in_maps = []
    for c in range(NC_):
        sl = slice(c * BPC, (c + 1) * BPC)
        m = dict(shared)
        m["hsT"] = np.ascontiguousarray(hs[sl].transpose(2, 0, 1).reshape(H, T))
        m["encT"] = np.ascontiguousarray(
            enc[sl].transpose(2, 0, 1).reshape(E, T)).astype(BF16NP)
        m["tembT"] = np.ascontiguousarray(temb[sl].T)
        in_maps.append(m)

    res = run_bass_kernel_spmd(nc, in_maps, core_ids=list(range(NC_)),
                               trace=_trace)
    out = np.empty((B, S, H), np.float32)
    for c in range(NC_):
        o = res.results[c]["outT"]  # [H, T]
        out[c * BPC:(c + 1) * BPC] = np.ascontiguousarray(o.T).reshape(BPC, S, H)
    if _trace:
        kernel.last_results = res
    return out


# revision 24
# speedup vs baseline: 1.0001x; 1.0001x over previous
"""BLIP3o DiT block on 8 Trainium2 NeuronCores.

Strategy: data-parallel over batch (32 batches -> 4 per core), zero collectives.
On-chip layout: activations live transposed [feature, token]; all matmul
operands are bf16 (1 cycle/row on the PE, half-size LDWEIGHTS, half HBM
traffic), accumulation stays fp32 in PSUM.  Residual stream (hsT),
modulation, rms statistics and rope tables stay fp32.

Overlap structure: each rms-norm's square+mean-square matmuls are issued
inside the preceding phase's consumers (mod loop for rms1, o1 for rms2, o2
for rms3) so only the short sqrt->recip->normalize tail sits between phases.
The final gate/residual/store is folded into the down1 PSUM drain.  Weight
stationaries are reused across both token halves (f-outer loops).

Per-core dataflow (T = 4*256 = 1024 tokens):
  modT  = (silu(temb) @ ada_w + ada_b)^T          (tiny-N matmuls, o-chunked)
  x1T   = rope(rms(hsT)*(1+sc_msa)+sh_msa)        (bf16)
  V1    = x1T-stationary @ wv1 (natural layout)
  QT,KT = wq1/wk1-stationary @ x1T
  attn1 : scoresT = KT-slice-stationary @ QT (both kc chunks into one PSUM
          bank); single exp per head on ScalarE -> bf16; AV + ones-denominator
  h1    = hsT + g_msa * (wo1-proj of attnout)      (in place)
  evaT  = eva_w-stationary @ encT + eva_b          (encT streamed in halves)
  attn2 : k2/v2 from evaT ; q2 from rms(h1,n2) ; no gate on residual
  yT    = rms(h2,n3)*(1+sc_mlp)+sh_mlp
  mlp   = (silu(gateT) * upT) @ down               (I split in halves)
  out   = h2 + g_mlp * mlp                         (folded into down1 drain)
"""
import os
import sys
import numpy as np

if "/root/pylocal" not in sys.path:
    sys.path.insert(0, "/root/pylocal")  # antenv.axon_hooks shim (NTFF tracing)
try:
    import antenv
    if "/root/pylocal/antenv" not in list(antenv.__path__):
        antenv.__path__.append("/root/pylocal/antenv")
except Exception:
    pass

import concourse.bass as bass
from concourse import bacc
import concourse.mybir as mybir
from concourse.tile import TileContext
from concourse.bass_utils import run_bass_kernel_spmd

F32 = mybir.dt.float32
BF16 = mybir.dt.bfloat16
AF = mybir.ActivationFunctionType
OP = mybir.AluOpType
BF16NP = mybir.dt.np(mybir.dt.bfloat16)

B, S, L, H, NH, HD, I, E = 32, 256, 256, 1024, 16, 64, 4096, 4096
EPS = 1e-6
GRID = 16
NC_ = 8            # cores
BPC = B // NC_     # batches per core = 4
T = BPC * S        # tokens per core = 1024
HC = H // 128      # 8 feature chunks
EC = E // 128      # 32
IC = I // 128      # 32
NCONST = 3 * HC + HC + 48 + 1 + 192   # n1T n2T n3T eva_bT ada_bT eps ada_bT_x4


def _rope_tables():
    q = H // 4
    inv = 1.0 / (10000.0 ** (np.arange(0, q, dtype=np.float64)[::2] / q))  # [128]
    qd = 128
    pos_x = np.repeat(np.arange(GRID, dtype=np.float64), GRID)  # [S]
    pos_y = np.tile(np.arange(GRID, dtype=np.float64), GRID)
    fx = pos_x[:, None] * inv[None, :qd]   # [S, 128]
    fy = pos_y[:, None] * inv[None, :qd]
    t = lambda a: np.ascontiguousarray(
        np.tile(a.T.astype(np.float32), (1, BPC)))  # [128, S] -> [128, T]
    return t(np.cos(fx)), t(np.sin(fx)), t(np.cos(fy)), t(np.sin(fy))


def build_program():
    nc = bacc.Bacc()

    # ---------------- DRAM params ----------------
    d = {}
    def P(name, shape, dt, out=False):
        d[name] = nc.declare_dram_parameter(name, list(shape), dt, isOutput=out)
        return d[name]

    hsT_d = P("hsT", [H, T], F32)
    encT_d = P("encT", [E, T], BF16)
    P("tembT", [H, BPC], F32)
    for w in ["wq1", "wk1", "wv1", "wo1", "wq2", "wk2", "wv2", "wo2"]:
        P(w, [H, H], BF16)
    P("eva_w", [E, H], BF16)
    P("ada_w", [H, 6 * H], BF16)
    P("gate_w", [H, I], BF16)
    P("up_w", [H, I], BF16)
    P("down_w", [I, H], BF16)
    P("constsF", [128, NCONST], F32)   # n1T | n2T | n3T | eva_bT | ada_bT | eps
    for tb in ["cxt", "sxt", "cyt", "syt"]:
        P(tb, [128, T], F32)
    P("ones", [128, 128], BF16)
    outT_d = P("outT", [H, T], F32, out=True)

    r3 = lambda ap: ap.rearrange("(c p) t -> p c t", p=128)

    tc_cm = TileContext(nc)
    tc = tc_cm.__enter__()

    open_pools = {}

    def pool(name, bufs=1, side="left"):
        p = tc.alloc_tile_pool(name=name, bufs=bufs, side=side)
        open_pools[name] = p
        return p

    def free(name):
        open_pools.pop(name).release()

    # long-lived small pools
    wpool = pool("wstream", bufs=4)         # tag "w8": [128, 8, 128] slots
    sml = pool("sml", bufs=1)               # resid/gsil tags
    const = pool("const", bufs=1)
    ps_proj = tc.alloc_tile_pool(name="ps_proj", bufs=2, space="PSUM")
    ps_sc = tc.alloc_tile_pool(name="ps_sc", bufs=2, space="PSUM")
    ps_av = tc.alloc_tile_pool(name="ps_av", bufs=2, space="PSUM")

    # ---------------- constants (one DMA) + temb first ----------------
    tembT_sb = const.tile([128, HC, BPC], F32)
    nc.sync.dma_start(tembT_sb[:], d["tembT"].rearrange("(c p) b -> p c b", p=128))
    cst = const.tile([128, NCONST], F32)
    nc.sync.dma_start(cst[:], d["constsF"][:])
    n_sb = {"n1T": cst[:, 0:8], "n2T": cst[:, 8:16], "n3T": cst[:, 16:24],
            "eva_bT": cst[:, 24:32], "ada_bT": cst[:, 32:80]}
    eps_sb = cst[:, 80:81]
    ada_bx4 = cst[:, 81:81 + 192].rearrange("p (o b) -> p o b", b=BPC)
    ones_sb = const.tile([128, 128], BF16)
    nc.sync.dma_start(ones_sb[:], d["ones"][:])

    modT = const.tile([128, 48, BPC], F32)      # 6 splits x 8 chunks
    scale1 = const.tile([128, HC, BPC], F32)    # n1*(1+sc_msa)
    scale3 = const.tile([128, HC, BPC], F32)    # n3*(1+sc_mlp)

    p_hs = pool("p_hs")
    hsT = p_hs.tile([128, HC, T], F32)          # becomes h1T, then h2T in place
    hs_r = r3(hsT_d)
    p_rope = pool("p_rope")
    rope_t = {}
    for tb in ["cxt", "sxt", "cyt", "syt"]:
        rope_t[tb] = p_rope.tile([128, T], F32, name=tb + "_sb")

    ms_pools = {}

    # ---------------- rms machinery ----------------
    def rms_begin(name, x_sb, side="left"):
        """Returns (issue, finish).  issue(c) squares chunk c and accumulates
        the mean-square matmuls; call it from inside the preceding phase.
        finish(consumer) runs sqrt->recip->normalize and feeds consumer."""
        rtmp = pool("rtmp_" + name, side=side)
        ps_ms = tc.alloc_tile_pool(name="ps_ms_" + name, bufs=1, space="PSUM")
        ms_pools[name] = ps_ms
        ms = [ps_ms.tile([128, 512], F32, name=f"ms_{name}_{t}")
              for t in range(2)]

        def issue(c, on_vector=False):
            sq = rtmp.tile([128, T], BF16, tag="sq", bufs=2, name=f"sq_{name}")
            if on_vector:
                nc.vector.tensor_tensor(sq[:], x_sb[:, c], x_sb[:, c], OP.mult)
            else:
                nc.scalar.activation(sq[:], x_sb[:, c], AF.Square)
            for t in range(2):
                nc.tensor.matmul(ms[t][:], ones_sb[:],
                                 sq[:, t * 512:(t + 1) * 512],
                                 start=(c == 0), stop=(c == HC - 1))

        def finish_half(t, consumer):
            sroot = rtmp.tile([128, 512], F32, tag="sroot", bufs=2,
                              name=f"sroot_{name}")
            nc.scalar.activation(sroot[:], ms[t][:], AF.Sqrt,
                                 bias=eps_sb, scale=1.0 / H)
            invn = rtmp.tile([128, 512], F32, tag="invn", bufs=2,
                             name=f"invn_{name}")
            nc.vector.reciprocal_approx_fast(invn[:], sroot[:])
            for c in range(HC):
                xn = rtmp.tile([128, 512], F32, tag="xn", bufs=4,
                               name=f"xn_{name}")
                nc.vector.tensor_tensor(xn[:],
                                        x_sb[:, c, t * 512:(t + 1) * 512],
                                        invn[:], OP.mult)
                consumer(c, t, xn)

        def release():
            ms_pools.pop(name).release()
            free("rtmp_" + name)

        def finish(consumer):
            finish_half(0, consumer)
            finish_half(1, consumer)
            release()

        return issue, finish, finish_half, release

    # x1T / rope staging (rms1 consumers write here during the mod loop)
    p_x1 = pool("p_x1", side="right")
    x1T = p_x1.tile([128, HC, T], BF16)
    p_xm = pool("p_xm")
    xm = [p_xm.tile([128, T], F32, name=f"xm{i}") for i in range(4)]

    def rms1_consumer(c, t, xn):
        dst = xm[c] if c < 4 else x1T[:, c]
        for b2 in range(2):
            b = 2 * t + b2
            nc.vector.tensor_scalar(dst[:, b * S:(b + 1) * S],
                                    xn[:, b2 * S:(b2 + 1) * S],
                                    scale1[:, c, b:b + 1],
                                    modT[:, 0 + c, b:b + 1],
                                    OP.mult, OP.add)

    r1_issue, r1_finish, r1_half, r1_release = rms_begin("r1", hsT, side="right")

    # ---------------- phase 0: mod loop with interleaved input DMAs ----------
    # mod bias-adds are done in two bulk vector ops (ada_bx4); the per-pair
    # PSUM drain is a scalar copy so the vector queue never stalls the PE.
    rp_holder = {}

    def rope_pair(pi):
        rp = rp_holder["rp"]
        (i0, i1, ct, st) = [(0, 1, "cxt", "sxt"), (2, 3, "cyt", "syt")][pi]
        a, bb = xm[i0], xm[i1]
        t1 = rp.tile([128, T], F32, tag="t1", bufs=2, name="t1")
        t2 = rp.tile([128, T], F32, tag="t2", bufs=2, name="t2")
        nc.vector.tensor_tensor(t1[:], a[:], rope_t[ct][:], OP.mult)
        nc.gpsimd.tensor_tensor(t2[:], bb[:], rope_t[st][:], OP.mult)
        nc.vector.tensor_tensor(x1T[:, i0], t1[:], t2[:], OP.subtract)
        t3 = rp.tile([128, T], F32, tag="t1", bufs=2, name="t3")
        t4 = rp.tile([128, T], F32, tag="t2", bufs=2, name="t4")
        nc.gpsimd.tensor_tensor(t3[:], a[:], rope_t[st][:], OP.mult)
        nc.vector.tensor_tensor(t4[:], bb[:], rope_t[ct][:], OP.mult)
        nc.vector.tensor_tensor(x1T[:, i1], t3[:], t4[:], OP.add)

    with nc.named_scope("mod"):
        stemb = const.tile([128, HC, BPC], BF16)
        nc.scalar.activation(stemb[:], tembT_sb[:], AF.Silu)
        ada_r = r3(d["ada_w"])  # [128, 8, 6144]
        mp = None
        for o in range(48):
            if o < HC:
                nc.sync.dma_start(hsT[:, o], hs_r[:, o])
            elif o < 12:
                tb = ["cxt", "sxt", "cyt", "syt"][o - 8]
                nc.sync.dma_start(rope_t[tb][:], d[tb][:])
            wt = wpool.tile([128, HC, 128], BF16, tag="w8", name="ada_t")
            nc.sync.dma_start(wt[:], ada_r[:, :, o * 128:(o + 1) * 128])
            if o % 2 == 0:
                mp = ps_sc.tile([128, 2, BPC], F32, tag="sc", name="mod_ps")
            for f in range(HC):
                nc.tensor.matmul(mp[:, o % 2], wt[:, f], stemb[:, f],
                                 start=(f == 0), stop=(f == HC - 1))
            if o % 2 == 1:
                nc.scalar.copy(modT[:, o - 1:o + 1, :], mp[:])
            if 8 <= o < 16:
                r1_issue(o - 8, on_vector=True)
            if o == 15:
                nc.vector.tensor_tensor(modT[:, 0:16], modT[:, 0:16],
                                        ada_bx4[:, 0:16], OP.add)
                for c in range(HC):
                    nc.vector.tensor_scalar(scale1[:, c], modT[:, 8 + c], 1.0,
                                            n_sb["n1T"][:, c:c + 1],
                                            OP.add, OP.mult)
            elif o == 17:
                with nc.named_scope("rms1"):
                    r1_half(0, rms1_consumer)
            elif o == 25:
                with nc.named_scope("rms1"):
                    r1_half(1, rms1_consumer)
                r1_release()
            elif o == 33:
                with nc.named_scope("rope"):
                    rp_holder["rp"] = pool("p_ropetmp")
                    rope_pair(0)
            elif o == 41:
                with nc.named_scope("rope"):
                    rope_pair(1)
                    free("p_ropetmp")
                free("p_xm")
                free("p_rope")
        nc.vector.tensor_tensor(modT[:, 16:48], modT[:, 16:48],
                                ada_bx4[:, 16:48], OP.add)
        for c in range(HC):
            nc.vector.tensor_scalar(scale3[:, c], modT[:, 32 + c], 1.0,
                                    n_sb["n3T"][:, c:c + 1],
                                    OP.add, OP.mult)

    # ---------------- helpers ----------------
    def copy_px(idx, dst, src):
        nc.scalar.copy(dst, src)

    def proj_T(name, w_name, src_sb, KC, consumer, OC=HC, wtag="w8"):
        """Y^T accumulation: lhsT = weight chunks [128,KC,128], rhs = src
        [128,KC,T].  Stationary reused across both token halves."""
        w_r = r3(d[w_name])
        with nc.named_scope(name):
            for o in range(OC):
                wt = wpool.tile([128, KC, 128], BF16, tag=wtag, name=f"{name}_w")
                nc.sync.dma_start(wt[:], w_r[:, :, o * 128:(o + 1) * 128])
                for t in range(2):
                    p = ps_proj.tile([128, 512], F32, tag="proj", name=f"{name}_ps")
                    for f in range(KC):
                        nc.tensor.matmul(p[:], wt[:, f],
                                         src_sb[:, f, t * 512:(t + 1) * 512],
                                         start=(f == 0), stop=(f == KC - 1))
                    consumer(o, t, p)

    def copy_act(dst):
        def c(o, t, p):
            copy_px(o + t, dst[:, o, t * 512:(t + 1) * 512], p[:])
        return c

    def vnat(w_name, src_sb, dst_v, scope, side="left"):
        """V natural [toks(128-chunks), H] from src-stationary matmuls."""
        w_r = r3(d[w_name])  # [128, HC, H]
        wv = pool("wv_" + scope, bufs=(1 if scope == "v2" else 2), side=side)
        with nc.named_scope(scope):
            for oh in range(2):
                wt = wv.tile([128, HC, 512], BF16, tag="wvnat", name=f"{scope}_w")
                nc.sync.dma_start(wt[:], w_r[:, :, oh * 512:(oh + 1) * 512])
                for t in range(2 * BPC):
                    p = ps_proj.tile([128, 512], F32, tag="proj", name=f"{scope}_ps")
                    KC = src_sb.shape[1]
                    for f in range(KC):
                        nc.tensor.matmul(p[:], src_sb[:, f, t * 128:(t + 1) * 128],
                                         wt[:, f], start=(f == 0), stop=(f == KC - 1))
                    copy_px(oh + t, dst_v[:, t, oh * 512:(oh + 1) * 512], p[:])
        free("wv_" + scope)

    def attention(qt_sb, kt_sb, vp_sb, out_sb, scope):
        attnp = pool("attnp_" + scope, bufs=3, side="right")
        ps_den = tc.alloc_tile_pool(name="ps_den_" + scope, bufs=2, space="PSUM")
        with nc.named_scope(scope):
            for b in range(BPC):
                for hc in range(NH // 2):
                    # head pair (2*hc, 2*hc+1) lives at row groups 0-63 / 64-127
                    # of chunk hc; their score MMs use disjoint PE quadrants.
                    qs = [qt_sb[ho * 64:(ho + 1) * 64, hc, b * S:(b + 1) * S]
                          for ho in range(2)]
                    at = []
                    for ho in range(2):
                        sc_ps = ps_sc.tile([128, 2, S], F32, tag="sc",
                                           name="sc_ps")
                        for kc in range(2):
                            nc.tensor.matmul(
                                sc_ps[:, kc],
                                kt_sb[ho * 64:(ho + 1) * 64, hc,
                                      b * S + kc * 128: b * S + (kc + 1) * 128],
                                qs[ho], start=True, stop=True)
                        a = attnp.tile([128, 2, S], BF16, tag=f"attn{ho}",
                                       name="attn_sb")
                        nc.scalar.activation(a[:], sc_ps[:], AF.Exp,
                                             scale=float(HD) ** -0.5)
                        at.append(a)
                    for ho in range(2):
                        h = 2 * hc + ho
                        av = ps_av.tile([64, S], F32, tag="av", name="av_ps")
                        for kc in range(2):
                            nc.tensor.matmul(av[:],
                                             vp_sb[:, b * 2 + kc,
                                                   h * 64:(h + 1) * 64],
                                             at[ho][:, kc],
                                             start=(kc == 0), stop=(kc == 1))
                        den = ps_den.tile([64, S], F32, tag="den", name="den_ps")
                        for kc in range(2):
                            nc.tensor.matmul(den[:], ones_sb[:, 0:64],
                                             at[ho][:, kc],
                                             start=(kc == 0), stop=(kc == 1))
                        inv = attnp.tile([64, S], F32, tag="inv", name="inv_sb")
                        nc.vector.reciprocal_approx_fast(inv[:], den[:])
                        nc.vector.tensor_tensor(
                            out_sb[ho * 64:(ho + 1) * 64, hc, b * S:(b + 1) * S],
                            av[:], inv[:], OP.mult)
        ps_den.release()
        free("attnp_" + scope)

    # ---------------- phase A: V, Q, K, attention, o1 ------------------------
    p_vp = pool("p_vp")
    vp = p_vp.tile([128, 2 * BPC, NH * 64], BF16)
    vnat("wv1", x1T, vp, "v1")

    p_qt = pool("p_qt"); qt = p_qt.tile([128, HC, T], BF16)
    p_kt = pool("p_kt"); kt = p_kt.tile([128, HC, T], BF16)
    proj_T("q1", "wq1", x1T, HC, copy_act(qt))
    proj_T("k1", "wk1", x1T, HC, copy_act(kt))
    free("p_x1")

    p_ao = pool("p_ao", side="right")
    attnout = p_ao.tile([128, HC, T], BF16)
    attention(qt, kt, vp, attnout, "attn1")
    free("p_kt"); free("p_qt"); free("p_vp")

    p_r2 = pool("p_r2")
    rms2T = p_r2.tile([128, HC, T], BF16)
    r2_issue, _, r2_half, r2_release = rms_begin("r2", hsT, side="right")

    def resid_gated(g_split, rms_issue=None):
        def c(o, t, p):
            tg = sml.tile([128, 512], F32, tag="resid", name="resid_t")
            for b2 in range(2):
                b = t * 2 + b2
                nc.vector.tensor_scalar(tg[:, b2 * S:(b2 + 1) * S],
                                        p[:, b2 * S:(b2 + 1) * S],
                                        modT[:, g_split * 8 + o, b:b + 1],
                                        None, OP.mult)
            nc.vector.tensor_tensor(hsT[:, o, t * 512:(t + 1) * 512],
                                    hsT[:, o, t * 512:(t + 1) * 512],
                                    tg[:], OP.add)
            if t == 1 and rms_issue is not None:
                rms_issue(o)
        return c

    proj_T("o1", "wo1", attnout, HC, resid_gated(2, r2_issue))

    # rms2 tail right after o1; its vector work overlaps eva's PE stream
    def rms2_consumer(c, t, xn):
        nc.vector.tensor_scalar(rms2T[:, c, t * 512:(t + 1) * 512], xn[:],
                                n_sb["n2T"][:, c:c + 1], None, OP.mult)

    with nc.named_scope("rms2"):
        r2_half(0, rms2_consumer)   # hsT now holds h1
        r2_half(1, rms2_consumer)

    # ---------------- phase B: evaT = eva_w-stat @ encT + b ------------------
    p_eva = pool("p_eva")
    evaT = p_eva.tile([128, HC, T], BF16)
    enc_r = r3(encT_d)  # [128, 32, T]
    w_r_eva = r3(d["eva_w"])  # [128, 32, 1024]
    with nc.named_scope("eva"):
        p_enc = pool("p_enc", bufs=3)
        wev = pool("p_weva", bufs=2)
        for th in range(2):
            # two encT quarter-tiles per half, each DMA'd once and reused by
            # all 8 o-chunks; bufs=3 lets the next half's first quarter prefetch.
            enq = []
            for tq2 in range(2):
                ench = p_enc.tile([128, EC, 256], BF16, tag="ench", name="ench")
                tq = th * 2 + tq2
                nc.sync.dma_start(ench[:], enc_r[:, :, tq * 256:(tq + 1) * 256])
                enq.append(ench)
            for o in range(HC):
                wt = wev.tile([128, EC, 128], BF16, tag="weva", name="eva_w_t")
                nc.sync.dma_start(wt[:], w_r_eva[:, :, o * 128:(o + 1) * 128])
                for tq2 in range(2):
                    tq = th * 2 + tq2
                    p = ps_proj.tile([128, 256], F32, tag="proj", name="eva_ps")
                    for f in range(EC):
                        nc.tensor.matmul(p[:], wt[:, f], enq[tq2][:, f],
                                         start=(f == 0), stop=(f == EC - 1))
                    nc.vector.tensor_scalar(evaT[:, o, tq * 256:(tq + 1) * 256],
                                            p[:], n_sb["eva_bT"][:, o:o + 1],
                                            None, OP.add)
        free("p_weva")
        free("p_enc")
    r2_release()
    free("p_ao")

    # ---------------- phase C: cross attention (k2, v2, q2) ------------------
    p_ao2 = pool("p_ao2", side="right")
    attn2out = p_ao2.tile([128, HC, T], BF16)
    p_k2 = pool("p_k2", side="right"); k2t = p_k2.tile([128, HC, T], BF16)
    proj_T("k2", "wk2", evaT, HC, copy_act(k2t))

    p_v2 = pool("p_v2", side="right")
    vp2 = p_v2.tile([128, 2 * BPC, NH * 64], BF16)
    vnat("wv2", evaT, vp2, "v2", side="right")
    free("p_eva")

    p_q2 = pool("p_q2", side="right"); q2t = p_q2.tile([128, HC, T], BF16)
    proj_T("q2", "wq2", rms2T, HC, copy_act(q2t))
    free("p_r2")

    attention(q2t, k2t, vp2, attn2out, "attn2")
    free("p_q2"); free("p_v2"); free("p_k2")

    # ---------------- phase D: rms3 + MLP ------------------------------------
    p_dacc = pool("p_dacc")
    dacc = p_dacc.tile([128, HC, T], F32)
    p_y = pool("p_y")
    yT = p_y.tile([128, HC, T], BF16)
    r3_issue, _, r3_half, _ = rms_begin("r3", hsT, side="right")

    def resid_plain(o, t, p):
        nc.vector.tensor_tensor(hsT[:, o, t * 512:(t + 1) * 512],
                                hsT[:, o, t * 512:(t + 1) * 512], p[:], OP.add)
        if t == 1:
            r3_issue(o)

    proj_T("o2", "wo2", attn2out, HC, resid_plain)

    def rms3_consumer(c, t, xn):
        for b2 in range(2):
            b = 2 * t + b2
            nc.vector.tensor_scalar(yT[:, c, b * S:(b + 1) * S],
                                    xn[:, b2 * S:(b2 + 1) * S],
                                    scale3[:, c, b:b + 1],
                                    modT[:, 24 + c, b:b + 1],
                                    OP.mult, OP.add)

    with nc.named_scope("rms3"):
        r3_half(0, rms3_consumer)   # hsT now holds h2
        r3_half(1, rms3_consumer)

    gate_r = r3(d["gate_w"])  # [128, 8, 4096]
    up_r = r3(d["up_w"])
    down_r = r3(d["down_w"])  # [128, 32, 1024]
    out_r = r3(outT_d)
    IHC = IC // 2  # 16 I-chunks per half
    p_mlp = pool("p_mlp", side="right")
    wmlp = pool("p_wmlp", bufs=4)
    wdn = pool("p_wdown", bufs=2, side="right")
    for ih in range(2):
        mlpT = p_mlp.tile([128, IHC, T], BF16, tag="mlp", bufs=1, name="mlpT")
        with nc.named_scope(f"gateup{ih}"):
            for o in range(IHC):
                oc = ih * IHC + o
                wg = wmlp.tile([128, HC, 128], BF16, tag="w8", name="gate_w_t")
                nc.sync.dma_start(wg[:], gate_r[:, :, oc * 128:(oc + 1) * 128])
                wu = wmlp.tile([128, HC, 128], BF16, tag="w8", name="up_w_t")
                nc.sync.dma_start(wu[:], up_r[:, :, oc * 128:(oc + 1) * 128])
                for t in range(2):
                    pg = ps_proj.tile([128, 512], F32, tag="proj", name="g_ps")
                    for f in range(HC):
                        nc.tensor.matmul(pg[:], wg[:, f],
                                         yT[:, f, t * 512:(t + 1) * 512],
                                         start=(f == 0), stop=(f == HC - 1))
                    pu = ps_proj.tile([128, 512], F32, tag="proj", name="u_ps")
                    for f in range(HC):
                        nc.tensor.matmul(pu[:], wu[:, f],
                                         yT[:, f, t * 512:(t + 1) * 512],
                                         start=(f == 0), stop=(f == HC - 1))
                    gs = sml.tile([128, 512], BF16, tag="gsil", name="gsil")
                    nc.scalar.activation(gs[:], pg[:], AF.Silu)
                    nc.vector.tensor_tensor(mlpT[:, o, t * 512:(t + 1) * 512],
                                            gs[:], pu[:], OP.mult)
        with nc.named_scope(f"down{ih}"):
            for o in range(HC):
                wt = wdn.tile([128, IHC, 128], BF16, tag="wdown", name="down_w_t")
                nc.sync.dma_start(wt[:],
                                  down_r[:, ih * IHC:(ih + 1) * IHC,
                                         o * 128:(o + 1) * 128])
                for t in range(2):
                    p = ps_proj.tile([128, 512], F32, tag="proj", name="d_ps")
                    for f in range(IHC):
                        nc.tensor.matmul(p[:], wt[:, f],
                                         mlpT[:, f, t * 512:(t + 1) * 512],
                                         start=(f == 0), stop=(f == IHC - 1))
                    sl = slice(t * 512, (t + 1) * 512)
                    if ih == 0:
                        nc.vector.tensor_copy(dacc[:, o, sl], p[:])
                    else:
                        # fold the final out = h2 + g_mlp * mlp into the drain
                        nc.vector.tensor_tensor(dacc[:, o, sl],
                                                dacc[:, o, sl], p[:], OP.add)
                        for b2 in range(2):
                            b = 2 * t + b2
                            nc.vector.tensor_scalar(
                                dacc[:, o, b * S:(b + 1) * S],
                                dacc[:, o, b * S:(b + 1) * S],
                                modT[:, 40 + o, b:b + 1], None, OP.mult)
                        nc.vector.tensor_tensor(dacc[:, o, sl],
                                                dacc[:, o, sl],
                                                hsT[:, o, sl], OP.add)
                        nc.sync.dma_start(out_r[:, o, sl], dacc[:, o, sl])
    free("p_wdown")
    free("p_wmlp")
    free("p_mlp")
    free("p_y")

    for nm in reversed(list(open_pools)):
        free(nm)
    for p in list(ms_pools.values()):
        p.release()
    ps_av.release(); ps_sc.release(); ps_proj.release()
    tc_cm.__exit__(None, None, None)
    nc.compile()
    return nc


_CACHE = {}


def _get_program():
    if "nc" not in _CACHE:
        _CACHE["nc"] = build_program()
    return _CACHE["nc"]


def kernel(hidden_states, encoder_hidden_states, timestep_emb,
           wq1, wk1, wv1, wo1, wq2, wk2, wv2, wo2,
           eva_w, eva_b, ada_w, ada_b, gate_w, up_w, down_w, n1, n2, n3,
           _trace=False):
    nc = _get_program()
    f32 = lambda a: np.ascontiguousarray(np.asarray(a), dtype=np.float32)
    bf = lambda a: np.ascontiguousarray(np.asarray(a), dtype=np.float32).astype(BF16NP)

    cxt, sxt, cyt, syt = _rope_tables()
    colchunks = lambda v, n: np.asarray(v, np.float32).reshape(n, 128).T
    ada_bT = colchunks(ada_b, 48)
    constsF = np.concatenate([
        colchunks(n1, HC), colchunks(n2, HC), colchunks(n3, HC),
        colchunks(eva_b, HC), ada_bT,
        np.full((128, 1), EPS, np.float32),
        np.repeat(ada_bT, 4, axis=1)], axis=1)
    shared = dict(
        wq1=bf(wq1), wk1=bf(wk1), wv1=bf(wv1), wo1=bf(wo1),
        wq2=bf(wq2), wk2=bf(wk2), wv2=bf(wv2), wo2=bf(wo2),
        eva_w=bf(eva_w), ada_w=bf(ada_w), gate_w=bf(gate_w),
        up_w=bf(up_w), down_w=bf(down_w),
        constsF=np.ascontiguousarray(constsF),
        cxt=cxt, sxt=sxt, cyt=cyt, syt=syt,
        ones=np.ones((128, 128), BF16NP),
    )
    hs = f32(hidden_states)
    enc = f32(encoder_hidden_states)
    temb = f32(timestep_emb)

    in_maps = []
    for c in range(NC_):
        sl = slice(c * BPC, (c + 1) * BPC)
        m = dict(shared)
        m["hsT"] = np.ascontiguousarray(hs[sl].transpose(2, 0, 1).reshape(H, T))
        m["encT"] = np.ascontiguousarray(
            enc[sl].transpose(2, 0, 1).reshape(E, T)).astype(BF16NP)
        m["tembT"] = np.ascontiguousarray(temb[sl].T)
        in_maps.append(m)

    res = run_bass_kernel_spmd(nc, in_maps, core_ids=list(range(NC_)),
                               trace=_trace)
    out = np.empty((B, S, H), np.float32)
    for c in range(NC_):
        o = res.results[c]["outT"]  # [H, T]
        out[c * BPC:(c + 1) * BPC] = np.ascontiguousarray(o.T).reshape(BPC, S, H)
    if _trace:
        kernel.last_results = res
    return out
